# revision 25
# baseline (speedup 1.0000x reference)
"""Trainium2 Bass kernel for GPT-2 style attention block (B=2, S=2048, D=1024, H=16).

Sharding (8 cores): head-pair sharding — core c owns heads (c, c+8) for BOTH
batches. Every AllToAll slot j then maps to a distinct (batch=j//4, seq
quarter=j%4) receiver and every sender has real data for every slot: no
payload replication, no receiver-side batch select. Two 512KB AllToAlls (one
per owned head) pipeline behind attention; c_proj accumulation is split into
two k-passes so the second collective hides under the first half of c_proj.

Other choices: transposed-scores attention with softmax denominator folded
into the PV matmul via a ones-column in V; causal masking applied post-exp
with gpsimd affine_select (upper-triangle zeroing) so DVE does no mask adds;
diagonal score matmuls truncated to the live column range; reciprocal
broadcast via gpsimd partition_broadcast instead of a PE ones-matmul.

Compute dtype bf16 (fp32 PSUM accumulation); normalization in fp32.
"""
import sys
sys.path.insert(0, '/opt/trn_rl_repo')

import numpy as np
import ml_dtypes

import concourse.bass as bass
import concourse.mybir as mybir
import concourse.tile as tile
from concourse import bacc
from concourse.bass_utils import run_bass_kernel_spmd

B, S, D = 2, 2048, 1024
H, HD = 16, 64
NCORES = 8

F32 = mybir.dt.float32
BF16 = mybir.dt.bfloat16
ADD = mybir.AluOpType.add
MULT = mybir.AluOpType.mult
BYPASS = mybir.AluOpType.bypass
GE = mybir.AluOpType.is_ge
EXP = mybir.ActivationFunctionType.Exp

WARMUP_MM = 10


def _emit(nc, tc):
    xT = nc.dram_tensor("xT", [D, 2 * S], BF16, kind="ExternalInput").ap()
    w_qk = nc.dram_tensor("w_qk", [D, 256], BF16, kind="ExternalInput").ap()
    w_v = nc.dram_tensor("w_v", [D, 128], BF16, kind="ExternalInput").ap()
    w_p = nc.dram_tensor("w_p", [D, D], BF16, kind="ExternalInput").ap()
    bqk = nc.dram_tensor("bqk", [128, 2], F32, kind="ExternalInput").ap()
    beff = nc.dram_tensor("beff", [1, D], BF16, kind="ExternalInput").ap()
    out = nc.dram_tensor("out", [512, D], F32, kind="ExternalOutput").ap()

    a2a_in = [nc.dram_tensor(f"a2a_in{u}", [8, 64, 512], BF16) for u in range(2)]
    a2a_out = [nc.dram_tensor(f"a2a_out{u}", [8, 64, 512], BF16) for u in range(2)]

    from contextlib import ExitStack
    ctx = ExitStack()
    cst = ctx.enter_context(tc.tile_pool(name="cst", bufs=1))
    pw = ctx.enter_context(tc.tile_pool(name="pw", bufs=4, space="PSUM"))
    psc = ctx.enter_context(tc.tile_pool(name="psc", bufs=2, space="PSUM"))
    sb = ctx.enter_context(tc.tile_pool(name="sb", bufs=3))

    # ---- resident SBUF loads, batch-0 inputs first so qk/attend start early.
    # Descriptor issue spread over the three DMA-capable engines (sync,
    # scalar, gpsimd) — a single queue serializes at ~0.6us per descriptor.
    xT_sb = cst.tile([128, 8, 2 * S], BF16)
    wqk_sb = cst.tile([128, 8, 256], BF16)
    wv_sb = cst.tile([128, 8, 128], BF16)
    xT_r = xT.rearrange("(k p) n -> p k n", p=128)
    dmae = [nc.sync, nc.scalar, nc.gpsimd]
    for k in range(8):
        dmae[k % 3].dma_start(xT_sb[:, k, 0:1024], xT_r[:, k, 0:1024])
        dmae[(k + 1) % 3].dma_start(xT_sb[:, k, 1024:S], xT_r[:, k, 1024:S])
        dmae[(k + 2) % 3].dma_start(
            wqk_sb[:, k], w_qk.rearrange("(k p) n -> p k n", p=128)[:, k])
        dmae[k % 3].dma_start(
            wv_sb[:, k], w_v.rearrange("(k p) n -> p k n", p=128)[:, k])
    bqk_sb = cst.tile([128, 2], F32)
    nc.sync.dma_start(bqk_sb[:], bqk)
    for k in range(8):
        dmae[k % 2].dma_start(xT_sb[:, k, S:2 * S], xT_r[:, k, S:2 * S])
    wp_sb = cst.tile([128, 8, D], BF16)
    nc.sync.dma_start(wp_sb[:], w_p.rearrange("(k p) n -> p k n", p=128))
    beff_sb = cst.tile([1, D], BF16)
    nc.sync.dma_start(beff_sb[:], beff)
    ones_sb = cst.tile([1, 64], BF16)
    nc.vector.memset(ones_sb[:], 1.0)
    ones128 = cst.tile([1, 128], BF16)
    nc.vector.memset(ones128[:], 1.0)

    # PE warmer: dependency-free junk matmuls keep the array busy during the
    # input DMAs so HAM unthrottles before real work arrives
    wrow = sb.tile([1, 512], BF16, tag="wrow")
    nc.vector.memset(wrow[:], 1.0)
    warm_ps = pw.tile([128, 512], F32, tag="w", name="warm")
    for _ in range(WARMUP_MM):
        nc.tensor.matmul(warm_ps[0:64, :], ones_sb[:], wrow[:],
                         start=True, stop=True)

    # qkT [128, 2, 4096]: dim1 0=q^T (prescaled 1/8), 1=k^T; dim2 = b*2048+s.
    # partitions 0-63 = head c, 64-127 = head c+8
    qkT_sb = cst.tile([128, 2, 2 * S], BF16)

    def qk_proj(m, b):
        ps = {q: pw.tile([128, 512], F32, tag="w", name=f"qk{m}{b}_{q}")
              for q in range(4)}
        for k in range(8):
            for q in range(4):
                nc.tensor.matmul(
                    ps[q][:], wqk_sb[:, k, m * 128:(m + 1) * 128],
                    xT_sb[:, k, b * S + q * 512:b * S + (q + 1) * 512],
                    start=(k == 0), stop=(k == 7))
        for q in range(4):
            nc.vector.tensor_scalar(
                out=qkT_sb[:, m, b * S + q * 512:b * S + (q + 1) * 512],
                in0=ps[q][:], scalar1=bqk_sb[:, m:m + 1], scalar2=None, op0=ADD)

    # V with interleaved ones column: V_sb [128, 32, 2*65], t = b*16 + seq tile
    V_sb = cst.tile([128, 32, 2 * 65], BF16)

    def v_ones():
        nc.vector.memset(
            V_sb[:].rearrange("p m (h c) -> p m h c", c=65)[:, :, :, 64:65], 1.0)

    def v_piece(t):
        ps = pw.tile([128, 512], F32, tag="w", name=f"v{t}")
        for k in range(8):
            nc.tensor.matmul(
                ps[:, :128], xT_sb[:, k, t * 128:(t + 1) * 128], wv_sb[:, k, :],
                start=(k == 0), stop=(k == 7))
        nc.vector.tensor_copy(
            out=V_sb[:, t].rearrange("p (h c) -> p h c", c=65)[:, :, 0:64],
            in_=ps[:, :128].rearrange("p (h c) -> p h c", c=64))

    attnT_sb = cst.tile([128, 2, S], BF16)   # [2 heads x 64, b, queries]
    # gathered attnT for my 512 q rows, one tile per collective so c_proj
    # pass 1 doesn't pick up a false dependency on the second a2a's recv
    proj_lo = cst.tile([128, 4, 512], BF16)
    proj_hi = cst.tile([128, 4, 512], BF16)
    proj_u = [proj_lo, proj_hi]

    pend = []

    def flush_norm():
        while pend:
            pend.pop(0)()

    def attend(u, b, qt):
        po = 64 * u
        at = pw.tile([128, 512], F32, tag="w", name=f"at{u}{b}_{qt}")
        nkb = 4 * qt + 4

        def pv(gl, pt):
            for i, kb in enumerate(gl):
                rel = max(0, kb * 128 - qt * 512)
                nc.tensor.matmul(
                    at[0:65, rel:512], V_sb[:, b * 16 + kb, u * 65:(u + 1) * 65],
                    pt[:, i * 512 + rel:(i + 1) * 512],
                    start=(kb == 0), stop=(kb == nkb - 1))

        # software pipeline (depth 2): emit pairs p+1/p+2's score matmuls
        # before pair p's PV so the PE streams through the exp latency
        inflight = []
        first_pair = True
        for g0 in range(0, nkb, 2):
            gl = list(range(g0, min(g0 + 2, nkb)))
            rels = [max(0, kb * 128 - qt * 512) for kb in gl]
            diag = [kb * 128 >= qt * 512 for kb in gl]
            sc = psc.tile([128, 1024], F32, tag="sc")
            pt = sb.tile([128, 1024], BF16, tag="pt")
            for i, kb in enumerate(gl):
                rel = rels[i]
                nc.tensor.matmul(
                    sc[:, i * 512 + rel:(i + 1) * 512],
                    qkT_sb[po:po + 64, 1, b * S + kb * 128:b * S + (kb + 1) * 128],
                    qkT_sb[po:po + 64, 0,
                           b * S + qt * 512 + rel:b * S + (qt + 1) * 512],
                    start=True, stop=True)
            if not any(diag):
                w = len(gl) * 512
                nc.scalar.activation(out=pt[:, :w], in_=sc[:, :w], func=EXP)
            else:
                for i, kb in enumerate(gl):
                    rel = rels[i]
                    nc.scalar.activation(
                        out=pt[:, i * 512 + rel:(i + 1) * 512],
                        in_=sc[:, i * 512 + rel:(i + 1) * 512], func=EXP)
                    if diag[i]:
                        # zero probs where key > query (post-exp causal mask)
                        nc.gpsimd.affine_select(
                            out=pt[:, i * 512 + rel:(i + 1) * 512],
                            in_=pt[:, i * 512 + rel:(i + 1) * 512],
                            pattern=[[1, 512 - rel]], compare_op=GE, fill=0.0,
                            base=0, channel_multiplier=-1)
            if first_pair:
                # emit the previous qt's PE broadcast matmul here, after this
                # qt's first score pair: its DVE recip chain has had time to
                # finish, so the PE doesn't stall at the qt boundary
                flush_norm()
                first_pair = False
            inflight.append((gl, pt))
            if len(inflight) > 2:
                pv(*inflight.pop(0))
        for pair in inflight:
            pv(*pair)
        # stash unnormalized attn + 1/denominator now; the PE broadcast and
        # final multiply+send are deferred into the next attend (or flush)
        sl = attnT_sb[po:po + 64, b, qt * 512:(qt + 1) * 512]
        nc.vector.tensor_copy(out=sl, in_=at[0:64, :])
        den1 = sb.tile([1, 512], F32, tag="den1")
        nc.vector.tensor_copy(out=den1[:], in_=at[64:65, :])
        rec1 = sb.tile([1, 512], F32, tag="rec1")
        nc.vector.reciprocal_approx_fast(rec1[:], den1[:])
        rec1b = sb.tile([1, 512], BF16, tag="rec1b")
        nc.vector.tensor_copy(out=rec1b[:], in_=rec1[:])

        def finish(sl=sl, rec1b=rec1b, u=u, b=b, qt=qt):
            bc = pw.tile([128, 512], F32, tag="w", name=f"bc{u}{b}_{qt}")
            nc.tensor.matmul(bc[0:64, :], ones_sb[:], rec1b[:],
                             start=True, stop=True)
            nc.vector.tensor_tensor(sl, sl, bc[0:64, :], MULT)
            nc.sync.dma_start(a2a_in[u].ap()[b * 4 + qt], sl)
        pend.append(finish)

    def a2a_trigger(u):
        flush_norm()
        nc.gpsimd.collective_compute(
            "AllToAll", BYPASS, replica_groups=[list(range(NCORES))],
            ins=[a2a_in[u].ap().opt()], outs=[a2a_out[u].ap().opt()])

    def a2a_recv(u):
        # slot c of a2a u carries head (c + 8u): rows of Wp k-subtile
        # k' = 4u + g come from cores 2g (upper 64 rows) and 2g+1 (lower).
        # First k-subtile lands alone so c_proj's first matmuls start sooner.
        nc.sync.dma_start(proj_u[u][0:64, 0, :], a2a_out[u].ap()[0])
        nc.sync.dma_start(proj_u[u][64:128, 0, :], a2a_out[u].ap()[1])
        src = a2a_out[u].ap().rearrange("(a two) p n -> two p a n", two=2)
        nc.sync.dma_start(proj_u[u][0:64, 1:4, :], src[0, :, 1:4])
        nc.sync.dma_start(proj_u[u][64:128, 1:4, :], src[1, :, 1:4])

    # c_proj: 8 PSUM accumulators [m x n] alive across both k-passes
    cp_ps = None

    def c_proj_pass(p):
        nonlocal cp_ps
        if p == 0:
            cp_ps = []
            for m in range(2):
                cp_ps.append([pw.tile([128, 512], F32, tag="w", name=f"cp{m}{n}")
                              for n in range(2)])
            for m in range(2):
                t = psc.tile([128, 1024], F32, tag="sc", name=f"cp{2 + m}")
                cp_ps.append([t[:, 0:512], t[:, 512:1024]])
            # fold the output bias in via a contraction-1 ones matmul so no
            # separate DVE add is needed at the tail
            for m in range(4):
                for n in range(2):
                    nc.tensor.matmul(
                        cp_ps[m][n], ones128[:],
                        beff_sb[:, n * 512:(n + 1) * 512],
                        start=True, stop=False)
        for m in range(4):
            for ki in range(4):
                kk = 4 * p + ki
                for n in range(2):
                    nc.tensor.matmul(
                        cp_ps[m][n], proj_u[p][:, ki, m * 128:(m + 1) * 128],
                        wp_sb[:, kk, n * 512:(n + 1) * 512],
                        start=False, stop=(p == 1 and ki == 3))
            if p == 1:
                out_sb = sb.tile([128, D], F32, tag="out")
                # PSUM -> SBUF move on the (idle) scalar engine, off DVE
                for n in range(2):
                    nc.scalar.activation(
                        out=out_sb[:, n * 512:(n + 1) * 512], in_=cp_ps[m][n],
                        func=mybir.ActivationFunctionType.Copy)
                nc.sync.dma_start(out[m * 128:(m + 1) * 128, :], out_sb[:])

    # ---- emission order tuned for overlap ----
    qk_proj(0, 0)
    qk_proj(1, 0)
    v_ones()
    # v pieces interleaved with attend qt's: attend(b, qt) only needs V
    # seq-tiles <= 4*qt+3, so exp/DVE work starts sooner
    for t in range(8):
        v_piece(t)
    attend(0, 0, 0)
    for t in range(8, 12):
        v_piece(t)
    attend(0, 0, 1)
    for t in range(12, 16):
        v_piece(t)
    attend(0, 0, 2)
    qk_proj(0, 1)
    qk_proj(1, 1)
    attend(0, 0, 3)
    for t in range(16, 24):
        v_piece(t)
    attend(0, 1, 0)
    for t in range(24, 28):
        v_piece(t)
    attend(0, 1, 1)
    for t in range(28, 32):
        v_piece(t)
    attend(0, 1, 2)
    attend(0, 1, 3)
    a2a_trigger(0)
    for qt in range(4):
        attend(1, 0, qt)
    for qt in range(4):
        attend(1, 1, qt)
    a2a_trigger(1)
    a2a_recv(0)
    c_proj_pass(0)
    a2a_recv(1)
    c_proj_pass(1)

    ctx.close()


def build_nc():
    nc = bacc.Bacc("TRN2", target_bir_lowering=False, debug=False, num_devices=NCORES)
    with tile.TileContext(nc) as tc:
        _emit(nc, tc)
    nc.compile()
    return nc


def shard_inputs(hidden_states, c_attn_w, c_attn_b, c_proj_w, c_proj_b):
    x = np.asarray(hidden_states, np.float32)
    W = np.asarray(c_attn_w, np.float32)
    bqkv = np.asarray(c_attn_b, np.float32)
    Wp = np.asarray(c_proj_w, np.float32)
    bp = np.asarray(c_proj_b, np.float32)

    wq, wk, wv = W[:, :D] * 0.125, W[:, D:2 * D], W[:, 2 * D:]
    bq, bk, bv = bqkv[:D] * 0.125, bqkv[D:2 * D], bqkv[2 * D:]
    beff = (bp + bv @ Wp).reshape(1, D).astype(ml_dtypes.bfloat16)
    wp_bf = Wp.astype(ml_dtypes.bfloat16)
    xT_bf = np.ascontiguousarray(
        np.concatenate([x[0].T, x[1].T], axis=1)).astype(ml_dtypes.bfloat16)

    in_maps = []
    for c in range(NCORES):
        hs = [slice(64 * c, 64 * (c + 1)), slice(64 * (c + 8), 64 * (c + 9))]
        wq2 = np.concatenate([wq[:, h] for h in hs], axis=1)
        wk2 = np.concatenate([wk[:, h] for h in hs], axis=1)
        wv2 = np.concatenate([wv[:, h] for h in hs], axis=1)
        w_qk = np.concatenate([wq2, wk2], axis=1)
        bq2 = np.concatenate([bq[h] for h in hs])
        bk2 = np.concatenate([bk[h] for h in hs])
        bqk_t = np.stack([bq2, bk2], axis=1).astype(np.float32).copy()
        in_maps.append(dict(
            xT=xT_bf,
            w_qk=w_qk.astype(ml_dtypes.bfloat16),
            w_v=wv2.astype(ml_dtypes.bfloat16),
            w_p=wp_bf,
            bqk=bqk_t,
            beff=beff,
        ))
    return in_maps


def unshard(results):
    full = np.zeros((B, S, D), np.float32)
    for c in range(NCORES):
        b, r = divmod(c, 4)
        full[b, 512 * r:512 * (r + 1)] = results[c]["out"]
    return full


_NC = None


def kernel(**inputs):
    global _NC
    if _NC is None:
        _NC = build_nc()
    in_maps = shard_inputs(**inputs)
    res = run_bass_kernel_spmd(_NC, in_maps, core_ids=list(range(NCORES)))
    return unshard(res.results)


if __name__ == "__main__":
    import jax
    with jax.default_device(jax.devices("cpu")[0]):
        import reference
        inputs = {k: np.asarray(v) for k, v in reference.setup_inputs().items()}
        expected = np.asarray(reference.reference(**inputs))
    actual = kernel(**inputs)
    err = np.abs(actual - expected)
    print("max abs err:", err.max(), "rel:", err.max() / np.abs(expected).max())


# revision 26
# speedup vs baseline: 1.0009x; 1.0009x over previous
"""Trainium2 Bass kernel for GPT-2 style attention block (B=2, S=2048, D=1024, H=16).

Sharding (8 cores): head-pair sharding — core c owns heads (c, c+8) for BOTH
batches. Every AllToAll slot j then maps to a distinct (batch=j//4, seq
quarter=j%4) receiver and every sender has real data for every slot: no
payload replication, no receiver-side batch select. Two 512KB AllToAlls (one
per owned head) pipeline behind attention; c_proj accumulation is split into
two k-passes so the second collective hides under the first half of c_proj.

Other choices: transposed-scores attention with softmax denominator folded
into the PV matmul via a ones-column in V; causal masking applied post-exp
with gpsimd affine_select (upper-triangle zeroing) so DVE does no mask adds;
diagonal score matmuls truncated to the live column range; reciprocal
broadcast via gpsimd partition_broadcast instead of a PE ones-matmul.

Compute dtype bf16 (fp32 PSUM accumulation); normalization in fp32.
"""
import sys
sys.path.insert(0, '/opt/trn_rl_repo')

import numpy as np
import ml_dtypes

import concourse.bass as bass
import concourse.mybir as mybir
import concourse.tile as tile
from concourse import bacc
from concourse.bass_utils import run_bass_kernel_spmd

B, S, D = 2, 2048, 1024
H, HD = 16, 64
NCORES = 8

F32 = mybir.dt.float32
BF16 = mybir.dt.bfloat16
ADD = mybir.AluOpType.add
MULT = mybir.AluOpType.mult
BYPASS = mybir.AluOpType.bypass
GE = mybir.AluOpType.is_ge
EXP = mybir.ActivationFunctionType.Exp

WARMUP_MM = 10


def _emit(nc, tc):
    xT = nc.dram_tensor("xT", [D, 2 * S], BF16, kind="ExternalInput").ap()
    w_qk = nc.dram_tensor("w_qk", [D, 256], BF16, kind="ExternalInput").ap()
    w_v = nc.dram_tensor("w_v", [D, 128], BF16, kind="ExternalInput").ap()
    w_p = nc.dram_tensor("w_p", [D, D], BF16, kind="ExternalInput").ap()
    bqk = nc.dram_tensor("bqk", [128, 2], F32, kind="ExternalInput").ap()
    beff = nc.dram_tensor("beff", [1, D], BF16, kind="ExternalInput").ap()
    out = nc.dram_tensor("out", [512, D], F32, kind="ExternalOutput").ap()

    a2a_in = [nc.dram_tensor(f"a2a_in{u}", [8, 64, 512], BF16) for u in range(2)]
    a2a_out = [nc.dram_tensor(f"a2a_out{u}", [8, 64, 512], BF16) for u in range(2)]

    from contextlib import ExitStack
    ctx = ExitStack()
    cst = ctx.enter_context(tc.tile_pool(name="cst", bufs=1))
    pw = ctx.enter_context(tc.tile_pool(name="pw", bufs=4, space="PSUM"))
    psc = ctx.enter_context(tc.tile_pool(name="psc", bufs=2, space="PSUM"))
    sb = ctx.enter_context(tc.tile_pool(name="sb", bufs=3))

    # ---- resident SBUF loads, batch-0 inputs first so qk/attend start early.
    # Descriptor issue spread over the three DMA-capable engines (sync,
    # scalar, gpsimd) — a single queue serializes at ~0.6us per descriptor.
    xT_sb = cst.tile([128, 8, 2 * S], BF16)
    wqk_sb = cst.tile([128, 8, 256], BF16)
    wv_sb = cst.tile([128, 8, 128], BF16)
    xT_r = xT.rearrange("(k p) n -> p k n", p=128)
    dmae = [nc.sync, nc.scalar, nc.gpsimd]
    for k in range(8):
        dmae[k % 3].dma_start(xT_sb[:, k, 0:1024], xT_r[:, k, 0:1024])
        dmae[(k + 1) % 3].dma_start(xT_sb[:, k, 1024:S], xT_r[:, k, 1024:S])
        dmae[(k + 2) % 3].dma_start(
            wqk_sb[:, k], w_qk.rearrange("(k p) n -> p k n", p=128)[:, k])
        dmae[k % 3].dma_start(
            wv_sb[:, k], w_v.rearrange("(k p) n -> p k n", p=128)[:, k])
    bqk_sb = cst.tile([128, 2], F32)
    nc.sync.dma_start(bqk_sb[:], bqk)
    for k in range(8):
        dmae[k % 2].dma_start(xT_sb[:, k, S:2 * S], xT_r[:, k, S:2 * S])
    wp_sb = cst.tile([128, 8, D], BF16)
    nc.sync.dma_start(wp_sb[:], w_p.rearrange("(k p) n -> p k n", p=128))
    beff_sb = cst.tile([1, D], BF16)
    nc.sync.dma_start(beff_sb[:], beff)
    ones_sb = cst.tile([1, 64], BF16)
    nc.vector.memset(ones_sb[:], 1.0)
    ones128 = cst.tile([1, 128], BF16)
    nc.vector.memset(ones128[:], 1.0)

    # PE warmer: dependency-free junk matmuls keep the array busy during the
    # input DMAs so HAM unthrottles before real work arrives
    wrow = sb.tile([1, 512], BF16, tag="wrow")
    nc.vector.memset(wrow[:], 1.0)
    warm_ps = pw.tile([128, 512], F32, tag="w", name="warm")
    for _ in range(WARMUP_MM):
        nc.tensor.matmul(warm_ps[0:64, :], ones_sb[:], wrow[:],
                         start=True, stop=True)

    # qkT [128, 2, 4096]: dim1 0=q^T (prescaled 1/8), 1=k^T; dim2 = b*2048+s.
    # partitions 0-63 = head c, 64-127 = head c+8
    qkT_sb = cst.tile([128, 2, 2 * S], BF16)

    def qk_proj(m, b):
        ps = {q: pw.tile([128, 512], F32, tag="w", name=f"qk{m}{b}_{q}")
              for q in range(4)}
        for k in range(8):
            for q in range(4):
                nc.tensor.matmul(
                    ps[q][:], wqk_sb[:, k, m * 128:(m + 1) * 128],
                    xT_sb[:, k, b * S + q * 512:b * S + (q + 1) * 512],
                    start=(k == 0), stop=(k == 7))
        for q in range(4):
            nc.vector.tensor_scalar(
                out=qkT_sb[:, m, b * S + q * 512:b * S + (q + 1) * 512],
                in0=ps[q][:], scalar1=bqk_sb[:, m:m + 1], scalar2=None, op0=ADD)

    # V with interleaved ones column: V_sb [128, 32, 2*65], t = b*16 + seq tile
    V_sb = cst.tile([128, 32, 2 * 65], BF16)

    def v_ones():
        nc.vector.memset(
            V_sb[:].rearrange("p m (h c) -> p m h c", c=65)[:, :, :, 64:65], 1.0)

    def v_piece(t):
        ps = pw.tile([128, 512], F32, tag="w", name=f"v{t}")
        for k in range(8):
            nc.tensor.matmul(
                ps[:, :128], xT_sb[:, k, t * 128:(t + 1) * 128], wv_sb[:, k, :],
                start=(k == 0), stop=(k == 7))
        nc.vector.tensor_copy(
            out=V_sb[:, t].rearrange("p (h c) -> p h c", c=65)[:, :, 0:64],
            in_=ps[:, :128].rearrange("p (h c) -> p h c", c=64))

    attnT_sb = cst.tile([128, 2, S], BF16)   # [2 heads x 64, b, queries]
    # gathered attnT for my 512 q rows, one tile per collective so c_proj
    # pass 1 doesn't pick up a false dependency on the second a2a's recv
    proj_lo = cst.tile([128, 4, 512], BF16)
    proj_hi = cst.tile([128, 4, 512], BF16)
    proj_u = [proj_lo, proj_hi]

    pend = []

    def flush_norm():
        while pend:
            pend.pop(0)()

    def attend(u, b, qt):
        po = 64 * u
        at = pw.tile([128, 512], F32, tag="w", name=f"at{u}{b}_{qt}")
        nkb = 4 * qt + 4

        def pv(gl, pt):
            for i, kb in enumerate(gl):
                rel = max(0, kb * 128 - qt * 512)
                nc.tensor.matmul(
                    at[0:65, rel:512], V_sb[:, b * 16 + kb, u * 65:(u + 1) * 65],
                    pt[:, i * 512 + rel:(i + 1) * 512],
                    start=(kb == 0), stop=(kb == nkb - 1))

        # software pipeline (depth 2): emit pairs p+1/p+2's score matmuls
        # before pair p's PV so the PE streams through the exp latency
        inflight = []
        first_pair = True
        for g0 in range(0, nkb, 2):
            gl = list(range(g0, min(g0 + 2, nkb)))
            rels = [max(0, kb * 128 - qt * 512) for kb in gl]
            diag = [kb * 128 >= qt * 512 for kb in gl]
            sc = psc.tile([128, 1024], F32, tag="sc")
            pt = sb.tile([128, 1024], BF16, tag="pt")
            for i, kb in enumerate(gl):
                rel = rels[i]
                nc.tensor.matmul(
                    sc[:, i * 512 + rel:(i + 1) * 512],
                    qkT_sb[po:po + 64, 1, b * S + kb * 128:b * S + (kb + 1) * 128],
                    qkT_sb[po:po + 64, 0,
                           b * S + qt * 512 + rel:b * S + (qt + 1) * 512],
                    start=True, stop=True)
            if not any(diag):
                w = len(gl) * 512
                nc.scalar.activation(out=pt[:, :w], in_=sc[:, :w], func=EXP)
            else:
                for i, kb in enumerate(gl):
                    rel = rels[i]
                    nc.scalar.activation(
                        out=pt[:, i * 512 + rel:(i + 1) * 512],
                        in_=sc[:, i * 512 + rel:(i + 1) * 512], func=EXP)
                    if diag[i]:
                        # zero probs where key > query (post-exp causal mask)
                        nc.gpsimd.affine_select(
                            out=pt[:, i * 512 + rel:(i + 1) * 512],
                            in_=pt[:, i * 512 + rel:(i + 1) * 512],
                            pattern=[[1, 512 - rel]], compare_op=GE, fill=0.0,
                            base=0, channel_multiplier=-1)
            if first_pair:
                # emit the previous qt's PE broadcast matmul here, after this
                # qt's first score pair: its DVE recip chain has had time to
                # finish, so the PE doesn't stall at the qt boundary
                flush_norm()
                first_pair = False
            inflight.append((gl, pt))
            if len(inflight) > 2:
                pv(*inflight.pop(0))
        for pair in inflight:
            pv(*pair)
        # stash unnormalized attn + 1/denominator now; the PE broadcast and
        # final multiply+send are deferred into the next attend (or flush)
        sl = attnT_sb[po:po + 64, b, qt * 512:(qt + 1) * 512]
        nc.vector.tensor_copy(out=sl, in_=at[0:64, :])
        den1 = sb.tile([1, 512], F32, tag="den1")
        nc.vector.tensor_copy(out=den1[:], in_=at[64:65, :])
        rec1 = sb.tile([1, 512], F32, tag="rec1")
        nc.vector.reciprocal_approx_fast(rec1[:], den1[:])
        rec1b = sb.tile([1, 512], BF16, tag="rec1b")
        nc.vector.tensor_copy(out=rec1b[:], in_=rec1[:])

        def finish(sl=sl, rec1b=rec1b, u=u, b=b, qt=qt):
            bc = pw.tile([128, 512], F32, tag="w", name=f"bc{u}{b}_{qt}")
            nc.tensor.matmul(bc[0:64, :], ones_sb[:], rec1b[:],
                             start=True, stop=True)
            nc.vector.tensor_tensor(sl, sl, bc[0:64, :], MULT)
            nc.sync.dma_start(a2a_in[u].ap()[b * 4 + qt], sl)
        pend.append(finish)

    def a2a_trigger(u):
        flush_norm()
        nc.gpsimd.collective_compute(
            "AllToAll", BYPASS, replica_groups=[list(range(NCORES))],
            ins=[a2a_in[u].ap().opt()], outs=[a2a_out[u].ap().opt()])

    def a2a_recv(u):
        # slot c of a2a u carries head (c + 8u): rows of Wp k-subtile
        # k' = 4u + g come from cores 2g (upper 64 rows) and 2g+1 (lower).
        # First k-subtile lands alone so c_proj's first matmuls start sooner.
        nc.sync.dma_start(proj_u[u][0:64, 0, :], a2a_out[u].ap()[0])
        nc.sync.dma_start(proj_u[u][64:128, 0, :], a2a_out[u].ap()[1])
        src = a2a_out[u].ap().rearrange("(a two) p n -> two p a n", two=2)
        nc.sync.dma_start(proj_u[u][0:64, 1:4, :], src[0, :, 1:4])
        nc.sync.dma_start(proj_u[u][64:128, 1:4, :], src[1, :, 1:4])

    # c_proj: 8 PSUM accumulators [m x n] alive across both k-passes
    cp_ps = None

    def c_proj_pass(p):
        nonlocal cp_ps
        if p == 0:
            cp_ps = []
            for m in range(2):
                cp_ps.append([pw.tile([128, 512], F32, tag="w", name=f"cp{m}{n}")
                              for n in range(2)])
            for m in range(2):
                t = psc.tile([128, 1024], F32, tag="sc", name=f"cp{2 + m}")
                cp_ps.append([t[:, 0:512], t[:, 512:1024]])
            # fold the output bias in via a contraction-1 ones matmul so no
            # separate DVE add is needed at the tail
            for m in range(4):
                for n in range(2):
                    nc.tensor.matmul(
                        cp_ps[m][n], ones128[:],
                        beff_sb[:, n * 512:(n + 1) * 512],
                        start=True, stop=False)
        for m in range(4):
            for ki in range(4):
                kk = 4 * p + ki
                for n in range(2):
                    nc.tensor.matmul(
                        cp_ps[m][n], proj_u[p][:, ki, m * 128:(m + 1) * 128],
                        wp_sb[:, kk, n * 512:(n + 1) * 512],
                        start=False, stop=(p == 1 and ki == 3))
            if p == 1:
                out_sb = sb.tile([128, D], F32, tag="out")
                # PSUM -> SBUF moves split across DVE and the scalar engine
                # so the 8 copies don't serialize ahead of the output DMAs
                nc.vector.tensor_copy(out=out_sb[:, 0:512], in_=cp_ps[m][0])
                nc.scalar.activation(
                    out=out_sb[:, 512:1024], in_=cp_ps[m][1],
                    func=mybir.ActivationFunctionType.Copy)
                nc.sync.dma_start(out[m * 128:(m + 1) * 128, :], out_sb[:])

    # ---- emission order tuned for overlap ----
    qk_proj(0, 0)
    qk_proj(1, 0)
    v_ones()
    # v pieces interleaved with attend qt's: attend(b, qt) only needs V
    # seq-tiles <= 4*qt+3, so exp/DVE work starts sooner
    for t in range(8):
        v_piece(t)
    attend(0, 0, 0)
    for t in range(8, 12):
        v_piece(t)
    attend(0, 0, 1)
    for t in range(12, 16):
        v_piece(t)
    attend(0, 0, 2)
    qk_proj(0, 1)
    qk_proj(1, 1)
    attend(0, 0, 3)
    for t in range(16, 24):
        v_piece(t)
    attend(0, 1, 0)
    for t in range(24, 28):
        v_piece(t)
    attend(0, 1, 1)
    for t in range(28, 32):
        v_piece(t)
    attend(0, 1, 2)
    attend(0, 1, 3)
    a2a_trigger(0)
    for qt in range(4):
        attend(1, 0, qt)
    for qt in range(4):
        attend(1, 1, qt)
    a2a_trigger(1)
    a2a_recv(0)
    c_proj_pass(0)
    a2a_recv(1)
    c_proj_pass(1)

    ctx.close()


def build_nc():
    nc = bacc.Bacc("TRN2", target_bir_lowering=False, debug=False, num_devices=NCORES)
    with tile.TileContext(nc) as tc:
        _emit(nc, tc)
    nc.compile()
    return nc


def shard_inputs(hidden_states, c_attn_w, c_attn_b, c_proj_w, c_proj_b):
    x = np.asarray(hidden_states, np.float32)
    W = np.asarray(c_attn_w, np.float32)
    bqkv = np.asarray(c_attn_b, np.float32)
    Wp = np.asarray(c_proj_w, np.float32)
    bp = np.asarray(c_proj_b, np.float32)

    wq, wk, wv = W[:, :D] * 0.125, W[:, D:2 * D], W[:, 2 * D:]
    bq, bk, bv = bqkv[:D] * 0.125, bqkv[D:2 * D], bqkv[2 * D:]
    beff = (bp + bv @ Wp).reshape(1, D).astype(ml_dtypes.bfloat16)
    wp_bf = Wp.astype(ml_dtypes.bfloat16)
    xT_bf = np.ascontiguousarray(
        np.concatenate([x[0].T, x[1].T], axis=1)).astype(ml_dtypes.bfloat16)

    in_maps = []
    for c in range(NCORES):
        hs = [slice(64 * c, 64 * (c + 1)), slice(64 * (c + 8), 64 * (c + 9))]
        wq2 = np.concatenate([wq[:, h] for h in hs], axis=1)
        wk2 = np.concatenate([wk[:, h] for h in hs], axis=1)
        wv2 = np.concatenate([wv[:, h] for h in hs], axis=1)
        w_qk = np.concatenate([wq2, wk2], axis=1)
        bq2 = np.concatenate([bq[h] for h in hs])
        bk2 = np.concatenate([bk[h] for h in hs])
        bqk_t = np.stack([bq2, bk2], axis=1).astype(np.float32).copy()
        in_maps.append(dict(
            xT=xT_bf,
            w_qk=w_qk.astype(ml_dtypes.bfloat16),
            w_v=wv2.astype(ml_dtypes.bfloat16),
            w_p=wp_bf,
            bqk=bqk_t,
            beff=beff,
        ))
    return in_maps


def unshard(results):
    full = np.zeros((B, S, D), np.float32)
    for c in range(NCORES):
        b, r = divmod(c, 4)
        full[b, 512 * r:512 * (r + 1)] = results[c]["out"]
    return full


_NC = None


def kernel(**inputs):
    global _NC
    if _NC is None:
        _NC = build_nc()
    in_maps = shard_inputs(**inputs)
    res = run_bass_kernel_spmd(_NC, in_maps, core_ids=list(range(NCORES)))
    return unshard(res.results)


if __name__ == "__main__":
    import jax
    with jax.default_device(jax.devices("cpu")[0]):
        import reference
        inputs = {k: np.asarray(v) for k, v in reference.setup_inputs().items()}
        expected = np.asarray(reference.reference(**inputs))
    actual = kernel(**inputs)
    err = np.abs(actual - expected)
    print("max abs err:", err.max(), "rel:", err.max() / np.abs(expected).max())
